# revision 37
# baseline (speedup 1.0000x reference)
"""Trainium2 Bass kernel for causal self-attention (B=4, T=2048, C=1024, H=16).

Sharding: 8 cores = 4 batches (data-parallel) x 2 head-groups (tensor-parallel,
8 heads each). Each core computes QKV for its heads, flash-style causal
attention, and a partial output projection over its half of the channels;
a pairwise ReduceScatter over output channels combines the two partials per
batch (output stays transposed [C, T]; host transposes back).

All operand transposes are done host-side (free), so every device DMA is a
plain contiguous copy. Attention is emitted one head at a time on a 3-slot
PSUM score pipeline; softmax (ACT engine) paces it, and independent QKV /
output-projection quanta are pulled from a work queue into the PE bubbles
(a debt counter tracks the ACT-vs-PE imbalance).

Self-contained: hardcodes shapes; host side only transposes/slices numpy
arrays.
"""

import ml_dtypes
import numpy as np
from contextlib import ExitStack

import concourse.bass as bass
import concourse.tile as tile
from concourse import bacc, mybir
from concourse.bass_utils import run_bass_kernel_spmd
from concourse.masks import make_identity, make_upper_triangular

F32 = mybir.dt.float32
BF16 = mybir.dt.bfloat16
AF = mybir.ActivationFunctionType
ALU = mybir.AluOpType

B, T, C = 4, 2048, 1024
H, HD = 16, 64
G = 2                    # tensor-parallel head groups
HL = H // G              # heads per core (8)
FL = HL * HD             # local q/k/v feature width (512)
N_CORES = 8
REPLICA_GROUPS = [[2 * b, 2 * b + 1] for b in range(B)]


def _make_pools(tc, ctx):
    p = {}
    p["consts"] = ctx.enter_context(tc.tile_pool(name="consts", bufs=1))
    p["tsb"] = ctx.enter_context(tc.tile_pool(name="tsb", bufs=2))
    p["pp"] = ctx.enter_context(tc.tile_pool(name="pp", bufs=4))
    p["rp"] = ctx.enter_context(tc.tile_pool(name="rp", bufs=4))
    p["psMM"] = ctx.enter_context(tc.tile_pool(name="psMM", bufs=3, space="PSUM"))
    p["psY"] = ctx.enter_context(tc.tile_pool(name="psY", bufs=2, space="PSUM"))
    p["dram"] = ctx.enter_context(tc.tile_pool(name="dram", bufs=1, space="DRAM"))
    return p


def _segments(qoff, width):
    """Split [qoff, width) at multiples of 512 (PSUM bank boundary)."""
    segs = []
    a = qoff
    while a < width:
        b = min((a // 512 + 1) * 512, width)
        segs.append((a, b))
        a = b
    return segs


def _alloc_tiles(p, t_seq):
    """Allocate all persistent tiles once; shared by prologue and bodies."""
    CT = C // 128
    TT = t_seq // 128
    TJ = t_seq // 1024
    FT = FL // 128
    OT = C // 128
    PT = FL // 128
    consts = p["consts"]
    tl = {}
    tl["ident_b"] = consts.tile([128, 128], BF16, tag="identb", name="identb")
    tl["negtri"] = consts.tile([128, 128], BF16, tag="negtri", name="negtri")
    tl["ones1"] = consts.tile([1, 128], BF16, tag="ones1", name="ones1")
    tl["bq_t"] = consts.tile([128, FT], F32, tag="bq", name="bq")
    tl["bk_t"] = consts.tile([128, FT], F32, tag="bk", name="bk")
    tl["bp_t"] = consts.tile([128, OT], F32, tag="bp", name="bp")
    tl["bv_b"] = consts.tile([1, FL], BF16, tag="bvb", name="bvb")
    tl["xT"] = [consts.tile([128, t_seq], BF16, tag=f"xT{i}", name=f"xT{i}") for i in range(CT)]
    tl["wqT"] = consts.tile([128, CT, FL], BF16, tag="wqT", name="wqT")
    tl["wkT"] = consts.tile([128, CT, FL], BF16, tag="wkT", name="wkT")
    tl["wvT"] = consts.tile([128, CT, FL], BF16, tag="wvT", name="wvT")
    tl["wpT"] = consts.tile([128, PT, C], BF16, tag="wpT", name="wpT")
    tl["QT"] = [[consts.tile([128, 1024], BF16, tag=f"QT{i}_{j}", name=f"QT{i}_{j}")
                 for j in range(TJ)] for i in range(FT)]
    tl["KT"] = [[consts.tile([128, 1024], BF16, tag=f"KT{i}_{j}", name=f"KT{i}_{j}")
                 for j in range(TJ)] for i in range(FT)]
    tl["Vt"] = [consts.tile([128, HL * 65], BF16, tag=f"Vt{i}", name=f"Vt{i}") for i in range(TT)]
    tl["yT"] = [consts.tile([128, FT, 1024], BF16, tag=f"yT{i}", name=f"yT{i}") for i in range(TJ)]
    return tl


def _emit_body(nc, tc, p, io, tl, t_seq, collective=True, upto='E', loads_mode='pre'):
    """Emit one full forward pass. t_seq: sequence length (2048 or 1024).

    loads_mode: 'pre' emits constants+input loads at the top (single-shot
    path); 'post' assumes a prologue already loaded them and re-emits the
    input loads at the END of the body (prefetch for the next loop
    iteration, overlapping attention/projection)."""
    CT = C // 128          # contraction tiles (8)
    TT = t_seq // 128      # token 128-tiles
    TJ = t_seq // 1024     # token 1024-chunks
    FT = FL // 128         # local f 128-tiles for q/k (4)
    OT = C // 128          # output-channel tiles (8)
    PT = FL // 128         # wp c_loc tiles (4)

    ident_b, negtri, ones1 = tl["ident_b"], tl["negtri"], tl["ones1"]
    bq_t, bk_t, bp_t, bv_b = tl["bq_t"], tl["bk_t"], tl["bp_t"], tl["bv_b"]
    xT, wqT, wkT, wvT, wpT = tl["xT"], tl["wqT"], tl["wkT"], tl["wvT"], tl["wpT"]
    QT, KT, Vt, yT = tl["QT"], tl["KT"], tl["Vt"], tl["yT"]

    if loads_mode == 'pre':
        make_identity(nc, ident_b)
        make_upper_triangular(nc, negtri, val=-50.0, diag=False)
        nc.vector.memset(ones1, 1.0)

    def emit_bias_loads():
        nc.sync.dma_start(bq_t, io["bqs"])
        nc.sync.dma_start(bk_t, io["bk"])
        nc.sync.dma_start(bp_t, io["bph"])
        nc.sync.dma_start(bv_b, io["bv"].rearrange("(a f) -> a f", a=1))  # bf16

    def emit_input_loads():
        # everything pre-transposed on host; contiguous DMAs
        emit_bias_loads()
        for ct in range(CT):
            nc.sync.dma_start(xT[ct], io["x"][ct * 128:(ct + 1) * 128, :])
        for ct in range(CT):
            nc.sync.dma_start(wqT[:, ct, :], io["wq"][ct * 128:(ct + 1) * 128, :])
            nc.sync.dma_start(wkT[:, ct, :], io["wk"][ct * 128:(ct + 1) * 128, :])
            nc.sync.dma_start(wvT[:, ct, :], io["wv"][ct * 128:(ct + 1) * 128, :])
        for ci in range(PT):
            nc.sync.dma_start(wpT[:, ci, :], io["wp"][ci * 128:(ci + 1) * 128, :])

    if loads_mode == 'pre':
        for tt in range(TT):  # ones columns of V (v_tile writes only [0:64])
            nc.vector.memset(Vt[tt].rearrange("p (h e) -> p h e", h=HL)[:, :, 64:65], 1.0)
        emit_input_loads()

    ccT = p["dram"].tile([C, t_seq], BF16, tag="ccT")
    direct_out = not collective

    if upto == 'L':       # prologue: constants + loads only
        return
    if upto == 'A':
        if loads_mode == 'post':
            emit_input_loads()
        nc.sync.dma_start(io["out"], ccT[0:C // 2, :])
        return

    # ---- QKV projection quanta (one 512-col half each; ~1.7us of PE) ----
    def qk_half(dst, w_T, bias, tj, ft, half, eng=None):
        def run():
            ps = p["psMM"].tile([128, 512], F32, tag="psMM", name="psMM")
            for ct in range(CT):
                nc.tensor.matmul(
                    ps,
                    lhsT=w_T[:, ct, ft * 128:(ft + 1) * 128],
                    rhs=xT[ct][:, tj * 1024 + half * 512: tj * 1024 + (half + 1) * 512],
                    start=(ct == 0),
                    stop=(ct == CT - 1),
                )
            dst_ap = dst[ft][tj][:, half * 512:(half + 1) * 512]
            if eng == 'act':
                nc.scalar.activation(dst_ap, ps, AF.Identity, bias=bias[:, ft:ft + 1])
            else:
                nc.vector.tensor_scalar(
                    dst_ap, ps, scalar1=bias[:, ft:ft + 1], scalar2=None, op0=ALU.add,
                )
        return run

    def v_tile(tt, eng=None):
        def run():
            ps = p["psMM"].tile([128, 512], F32, tag="psMM", name="psMM")
            for ct in range(CT):
                nc.tensor.matmul(
                    ps,
                    lhsT=xT[ct][:, tt * 128:(tt + 1) * 128],
                    rhs=wvT[:, ct, :],
                    start=(ct == 0),
                    stop=False,
                )
            nc.tensor.matmul(ps, lhsT=ones1, rhs=bv_b, start=False, stop=True)
            dst_ap = Vt[tt].rearrange("p (h e) -> p h e", h=HL)[:, :, 0:64]
            if eng == 'act':
                nc.scalar.activation(dst_ap, ps, AF.Identity)
            else:
                nc.vector.tensor_copy(dst_ap, ps)
        return run

    def qkv_items(tj, eng=None):
        """Work items (deadline_key, closure); forced before their consumer."""
        items = []
        for ft in range(FT):
            for half in range(2):
                items.append((('K', ft, tj), qk_half(KT, wkT, bk_t, tj, ft, half, eng)))
        for tt in range(tj * 8, tj * 8 + 8):
            items.append((('V', tt), v_tile(tt, eng)))
        for ft in range(FT):
            for half in range(2):
                items.append((('Q', ft, tj), qk_half(QT, wqT, bq_t, tj, ft, half, eng)))
        return items

    # ---- output-projection half quanta (~0.9us of PE each) ----
    def proj_half(tj, ot, half, box, eng=None):
        def run():
            if box[0] is None:
                box[0] = p["tsb"].tile([128, 1024], BF16, tag="tsb", name="tsb")
            tsb = box[0]
            ps = p["psMM"].tile([128, 512], F32, tag="psMM", name="psMM")
            for ci in range(PT):
                nc.tensor.matmul(
                    ps,
                    lhsT=wpT[:, ci, ot * 128:(ot + 1) * 128],
                    rhs=yT[tj][:, ci, half * 512:(half + 1) * 512],
                    start=(ci == 0),
                    stop=(ci == PT - 1),
                )
            nc.vector.tensor_scalar(
                tsb[:, half * 512:(half + 1) * 512], ps,
                scalar1=bp_t[:, ot:ot + 1], scalar2=None, op0=ALU.add,
            )
            if half == 1:
                nc.sync.dma_start(
                    ccT[ot * 128:(ot + 1) * 128, tj * 1024:(tj + 1) * 1024], tsb
                )
                if direct_out and ot < OT // 2:
                    # non-collective stand-in for the final store: write this
                    # core's partial straight from SBUF (no DRAM->DRAM copy)
                    nc.sync.dma_start(
                        io["out"][ot * 128:(ot + 1) * 128, tj * 1024:(tj + 1) * 1024],
                        tsb,
                    )
        return run

    def proj_items(tj, ots, eng=None):
        qs = []
        for ot in ots:
            box = [None]
            qs.append((None, proj_half(tj, ot, 0, box, eng)))
            qs.append((None, proj_half(tj, ot, 1, box, eng)))
        return qs

    # ---- attention: one head at a time, ACT-paced, fillers in the bubbles ----
    def emit_attn_head(jq, h, pull, force, cadence):
        ftq, po = h // 2, (h % 2) * 64
        ni = 8 * (jq + 1)
        last_a = 8 * jq + 3
        force(('Q', ftq, jq))
        ypA = p["psY"].tile([65, 512], F32, tag="psY", name=f"ypA{h}")
        ypB = p["psY"].tile([65, 512], F32, tag="psY", name=f"ypB{h}")

        def emit_av(prev):
            i, pt, qoff = prev
            force(('V', i))
            for (a, b) in _segments(qoff, 1024):
                yp, off, lst = (ypA, 0, last_a) if a < 512 else (ypB, 512, ni - 1)
                nc.tensor.matmul(
                    yp[:, a - off:b - off],
                    lhsT=Vt[i][:, h * 65:h * 65 + 65],
                    rhs=pt[:, a:b],
                    start=(i == 0),
                    stop=(i == lst),
                )

        prev = None
        for i in range(ni):
            if i % 8 == 0:
                force(('K', ftq, i // 8))
            q0 = max(jq * 1024, i * 128)
            qoff = q0 - jq * 1024
            diag = i * 128 >= jq * 1024
            w = 1024 - qoff
            sp = p["psMM"].tile([128, 1024], F32, tag="psMM", name="psMM")
            for si, (a, b) in enumerate(_segments(qoff, 1024)):
                nc.tensor.matmul(
                    sp[:, a:b],
                    lhsT=KT[ftq][i // 8][po:po + 64, (i % 8) * 128:(i % 8 + 1) * 128],
                    rhs=QT[ftq][jq][po:po + 64, a:b],
                    start=True,
                    stop=not (diag and si == 0),
                )
            if diag:
                nc.tensor.matmul(   # causal mask: -50 above the diagonal
                    sp[:, qoff:qoff + 128], lhsT=negtri, rhs=ident_b,
                    start=False, stop=True,
                )
            pt = p["pp"].tile([128, 1024], BF16, tag="pp", name="pp")
            nc.scalar.activation(pt[:, qoff:1024], sp[:, qoff:1024], AF.Exp)
            if prev is not None:
                emit_av(prev)
            prev = (i, pt, qoff)
            if i % cadence == cadence - 1:
                pull(1)
        emit_av(prev)

        # evacuate accumulators early, then normalize: yT = yc[0:64]/yc[64]
        for yp, off in ((ypA, 0), (ypB, 512)):
            yc = p["rp"].tile([65, 512], F32, tag="yc", name="yc")
            nc.vector.tensor_copy(yc, yp)
            r = p["rp"].tile([1, 512], F32, tag="r", name="r")
            nc.vector.reciprocal(r, yc[64:65, :])
            R = p["rp"].tile([64, 512], F32, tag="R", name="R")
            nc.gpsimd.partition_broadcast(R, r)
            nc.vector.tensor_mul(
                yT[jq][po:po + 64, ftq, off:off + 512], yc[0:64, :], R
            )
        pull(1)

    if upto in ('B', 'C'):
        for tj in range(TJ):
            for _, q in qkv_items(tj):
                q()
        if upto == 'C':
            for tj in range(TJ):
                for h in range(HL):
                    emit_attn_head(tj, h, lambda n: 0, lambda k: None, 10 ** 6)
        nc.sync.dma_start(io["out"], ccT[0:C // 2, :])
        return

    # ---- pipelined emission ----
    work = []

    def pull(n):
        done = 0
        while done < n and work:
            work.pop(0)[1]()
            done += 1
        return done

    def force(key):
        i = 0
        while i < len(work):
            if work[i][0] == key:
                work.pop(i)[1]()
            else:
                i += 1

    for _, q in qkv_items(0):
        q()
    if TJ > 1:
        work.extend(qkv_items(1))
        for h in range(HL):
            emit_attn_head(0, h, pull, force, 4)
        work.extend(proj_items(0, range(OT)))
        for h in range(HL):
            emit_attn_head(1, h, pull, force, 2)
        while work:
            work.pop(0)[1]()
        for _, q in proj_items(1, range(OT)):
            q()
    else:
        for h in range(HL):
            emit_attn_head(0, h, pull, force, 4)
        for _, q in proj_items(0, range(OT)):
            q()

    if loads_mode == 'post':
        emit_input_loads()     # prefetch next iteration across the loop edge

    # ---- pairwise ReduceScatter over output channels + final store ----
    if collective:
        cc_out = p["dram"].tile([C // 2, t_seq], BF16, tag="cc_out")
        nc.gpsimd.collective_compute(
            "ReduceScatter",
            ALU.add,
            replica_groups=REPLICA_GROUPS,
            ins=[ccT[:].opt()],
            outs=[cc_out[:].opt()],
        )
        nc.gpsimd.dma_start(io["out"], cc_out[:])


def build_program(t_seq=T, repeat=1, collective=True, upto='E', unroll=False):
    nc = bacc.Bacc("TRN2", target_bir_lowering=False, debug=False, num_devices=N_CORES)
    io = {
        "x": nc.dram_tensor("x", [C, t_seq], BF16, kind="ExternalInput").ap(),
        "wq": nc.dram_tensor("wq", [C, FL], BF16, kind="ExternalInput").ap(),
        "wk": nc.dram_tensor("wk", [C, FL], BF16, kind="ExternalInput").ap(),
        "wv": nc.dram_tensor("wv", [C, FL], BF16, kind="ExternalInput").ap(),
        "wp": nc.dram_tensor("wp", [FL, C], BF16, kind="ExternalInput").ap(),
        "bqs": nc.dram_tensor("bqs", [128, FL // 128], F32, kind="ExternalInput").ap(),
        "bk": nc.dram_tensor("bk", [128, FL // 128], F32, kind="ExternalInput").ap(),
        "bv": nc.dram_tensor("bv", [FL], BF16, kind="ExternalInput").ap(),
        "bph": nc.dram_tensor("bph", [128, C // 128], F32, kind="ExternalInput").ap(),
        "out": nc.dram_tensor("out", [C // 2, t_seq], BF16, kind="ExternalOutput").ap(),
    }
    with tile.TileContext(nc) as tc:
        with ExitStack() as ctx:
            pools = _make_pools(tc, ctx)
            tl = _alloc_tiles(pools, t_seq)
            if repeat == 1:
                _emit_body(nc, tc, pools, io, tl, t_seq, collective=collective, upto=upto)
            else:
                _emit_body(nc, tc, pools, io, tl, t_seq, collective=collective, upto='L',
                           loads_mode='pre')      # prologue: consts + loads
                u = 2 if repeat % 2 == 0 else 1
                body = lambda: _emit_body(nc, tc, pools, io, tl, t_seq,
                                          collective=collective, upto=upto,
                                          loads_mode='post')
                if unroll:   # sim-only: For_i doesn't run under no_exec CoreSim
                    for _ in range(repeat):
                        body()
                else:
                    with tc.For_i(0, repeat // u, 1) as _:
                        for _k in range(u):
                            body()
                    for _k in range(repeat % u):
                        body()
    nc.compile()
    return nc


def make_in_maps(x, w_attn, b_attn, w_proj, b_proj):
    x = np.asarray(x, dtype=np.float32)
    w_attn = np.asarray(w_attn, dtype=np.float32)
    b_attn = np.asarray(b_attn, dtype=np.float32)
    w_proj = np.asarray(w_proj, dtype=np.float32)
    b_proj = np.asarray(b_proj, dtype=np.float32)
    bf = ml_dtypes.bfloat16
    in_maps = []
    for c in range(N_CORES):
        b, g = c // 2, c % 2
        fs = slice(g * FL, (g + 1) * FL)
        # scale folded into wq/bq so Q comes out pre-scaled by 1/sqrt(hd)
        wq = w_attn[0 * C:][:C][fs] * np.float32(0.125)
        bq = b_attn[0 * C:][:C][fs] * np.float32(0.125)
        in_maps.append({
            "x": np.ascontiguousarray(x[b].T).astype(bf),
            "wq": np.ascontiguousarray(wq.T).astype(bf),
            "wk": np.ascontiguousarray(w_attn[1 * C:][:C][fs].T).astype(bf),
            "wv": np.ascontiguousarray(w_attn[2 * C:][:C][fs].T).astype(bf),
            "wp": np.ascontiguousarray(w_proj[:, fs].T).astype(bf),
            "bqs": np.ascontiguousarray(bq.reshape(-1, 128).T),
            "bk": np.ascontiguousarray(b_attn[1 * C:][:C][fs].reshape(-1, 128).T),
            "bv": np.ascontiguousarray(b_attn[2 * C:][:C][fs]).astype(bf),
            "bph": np.ascontiguousarray((b_proj * np.float32(0.5)).reshape(-1, 128).T),
        })
    return in_maps


_PROG = None


def kernel(x, w_attn, b_attn, w_proj, b_proj):
    global _PROG
    if _PROG is None:
        _PROG = build_program()
    in_maps = make_in_maps(x, w_attn, b_attn, w_proj, b_proj)
    res = run_bass_kernel_spmd(_PROG, in_maps, core_ids=list(range(N_CORES))).results
    out = np.empty((B, T, C), dtype=np.float32)
    for c in range(N_CORES):
        b, g = c // 2, c % 2
        out[b, :, g * (C // 2):(g + 1) * (C // 2)] = \
            res[c]["out"].astype(np.float32).T
    return out
